# revision 6
# baseline (speedup 1.0000x reference)
"""Trainium2 Bass kernel for the DPRNN block (channel-norm -> unfold ->
4x bidirectional SRU -> conv-transpose -> residual).

Sharding: data-parallel over the B*T=512 sequences; 64 sequences per core.
All weights replicated. Each core runs the full pipeline on its shard.

v2: mixed-precision fp8 (e4m3 DoubleRow) for L0 (all gates), L1-3 f/r gates
and the transposed conv; z/hp stay bf16 on L1-3. PSUM-resident z/hp
evacuation: b=(1-f)*z fused on DVE (affine_mul_reduce from PSUM), hp copied
by ScalarE, highway final add on GpSimd. Engine balance targets DVE
(scan-bound) at ~18us/unit.

Layout (per core): sequences live in 128-column blocks (121 valid SRU steps
+ 7 pad columns). Pads carry f=0, b=0 through the scan so a single
tensor_tensor_scan per (layer,half,dir) handles 32 sequences.
"""
import os
import numpy as np
import ml_dtypes

import concourse.bass as bass
import concourse.mybir as mybir
import concourse.tile as tile
from concourse import bacc
from concourse import bass_utils

F32 = mybir.dt.float32
BF16 = mybir.dt.bfloat16
FP8 = mybir.dt.float8e4
DR = mybir.MatmulPerfMode.DoubleRow
E4 = ml_dtypes.float8_e4m3
BFD = ml_dtypes.bfloat16

B, C, T, F_ = 4, 64, 128, 128
H, K = 128, 8
L = F_ - K + 1            # 121
EPS = 1e-8
NCORES = 8
NLOC = (B * T) // NCORES  # 64 sequences per core
NF = NLOC * 128           # 8192
HC8 = NF + 16             # fp8 block stride (16B aligned)
WS = 32.0                 # fp8 weight prescale (SRU layers)
SPAN = 1024

_CACHE = {}


def _build():
    nc = bacc.Bacc("TRN2", target_bir_lowering=False, debug=False)
    AF = mybir.ActivationFunctionType
    OP = mybir.AluOpType

    # ---------------- DRAM tensors ----------------
    u_d = nc.dram_tensor("u", [C, NLOC, F_], F32, kind="ExternalInput").ap()
    un_d = nc.dram_tensor("un", [NLOC, C, F_], F32, kind="ExternalInput").ap()
    w0_d = nc.dram_tensor("w0p", [2, 2, 128, 2 * 4 * 128], FP8, kind="ExternalInput").ap()
    wfr_d = nc.dram_tensor("wfr", [3, 2, 128, 2 * 2 * 128], FP8, kind="ExternalInput").ap()
    wzh_d = nc.dram_tensor("wzh", [3, 2, 2, 128, 2 * 128], BF16, kind="ExternalInput").ap()
    cw_d = nc.dram_tensor("cwp", [8, 128, 2 * 64], FP8, kind="ExternalInput").ap()
    bf_d = nc.dram_tensor("bfp", [4, 2, 128], F32, kind="ExternalInput").ap()
    br_d = nc.dram_tensor("brp", [4, 2, 128], F32, kind="ExternalInput").ap()
    gm_d = nc.dram_tensor("gm", [C], F32, kind="ExternalInput").ap()
    bt_d = nc.dram_tensor("bt", [C], F32, kind="ExternalInput").ap()
    cb_d = nc.dram_tensor("cb", [2 * C], F32, kind="ExternalInput").ap()
    out_d = nc.dram_tensor("o", [C, NF], F32, kind="ExternalOutput").ap()

    with tile.TileContext(nc) as tc:
        with tc.tile_pool(name="const", bufs=1) as cp:
            # ---- weights / biases resident in SBUF (DMAs issued first) ----
            w0_t = cp.tile([128, 2 * 2 * 2 * 4 * 128], FP8)
            w0_v = w0_t[:].rearrange("p (d j g o m) -> p d j g o m",
                                     d=2, j=2, g=2, o=4)
            nc.sync.dma_start(
                w0_v.rearrange("p d j g o m -> p d j (g o m)"),
                w0_d.rearrange("d j p q -> p d j q"))
            wfr_t = cp.tile([128, 3 * 2 * 2 * 2 * 128], FP8)
            wfr_v = wfr_t[:].rearrange("p (i d g o m) -> p i d g o m",
                                       i=3, d=2, g=2, o=2)
            nc.scalar.dma_start(
                wfr_v.rearrange("p i d g o m -> p i d (g o m)"),
                wfr_d.rearrange("i d p q -> p i d q"))
            wzh_t = cp.tile([128, 3 * 2 * 2 * 2 * 128], BF16)
            wzh_v = wzh_t[:].rearrange("p (i d ct o m) -> p i d ct o m",
                                       i=3, d=2, ct=2, o=2)
            nc.scalar.dma_start(
                wzh_v.rearrange("p i d ct o m -> p i d ct (o m)"),
                wzh_d.rearrange("i d ct p q -> p i d ct q"))
            cw_t = cp.tile([128, 8 * 2 * 64], FP8)
            cw_v = cw_t[:].rearrange("p (k g m) -> p k g m", k=8, g=2)
            nc.sync.dma_start(cw_v.rearrange("p k g m -> p k (g m)"),
                              cw_d.rearrange("k p q -> p k q"))
            bfp_t = cp.tile([128, 8], F32)
            nc.sync.dma_start(bfp_t[:].rearrange("p (i d) -> p i d", i=4),
                              bf_d.rearrange("i d p -> p i d"))
            brp_t = cp.tile([128, 8], F32)
            nc.sync.dma_start(brp_t[:].rearrange("p (i d) -> p i d", i=4),
                              br_d.rearrange("i d p -> p i d"))
            gm1_t = cp.tile([1, C], F32)
            nc.sync.dma_start(gm1_t[:], gm_d.rearrange("(a c) -> a c", a=1))
            bt_t = cp.tile([C, 1], F32)
            nc.sync.dma_start(bt_t[:], bt_d.rearrange("(c a) -> c a", a=1))
            cb_t = cp.tile([128, 1], F32)   # convb duplicated on both halves
            nc.sync.dma_start(cb_t[:], cb_d.rearrange("(c a) -> c a", a=1))

            # ---- long-lived activations ----
            # xn packed: rows 0:64 = channels, cols 0:4096 (global cols 0:4096)
            #            rows 64:128 = global cols 4096:8192
            xn_t = cp.tile([128, NF // 2], F32)
            # xn8: fp8 L0 input; block g in {0,1}: row p=c+64*s holds
            # xn[c, col + s + 2g]
            xn8_t = cp.tile([128, 2 * HC8], FP8)
            xn8_v = xn8_t[:].rearrange("p (g q) -> p g q", g=2)
            h_t = [cp.tile([128, NF], BF16, name=f"h{i}") for i in range(4)]
            h8_t = [cp.tile([128, 2 * HC8], FP8, name=f"h8{i}") for i in range(2)]
            h8_v = [t[:].rearrange("p (g q) -> p g q", g=2) for t in h8_t]
            acc_t = cp.tile([128, 1], F32)      # AMR reduce scratch
            A_t = cp.tile([NLOC, 128], F32)     # rstd
            B_t = cp.tile([NLOC, 128], F32)     # -mu*rstd

            # zero tails of xn8 (cols >= NF never written by chunks)
            nc.gpsimd.memset(xn8_v[:, :, NF:HC8], 0.0)
            nc.gpsimd.memset(xn8_v[:, 0, NF - 1:NF], 0.0)   # s=1 col 8191
            nc.gpsimd.memset(xn8_v[:, 1, NF - 3:NF], 0.0)   # blk B last cols

            # ================= channel norm: stats =================
            with (
                tc.tile_pool(name="normn", bufs=1) as nnp,
                tc.tile_pool(name="norms", bufs=1) as nsp,
            ):
                u_nn = nnp.tile([NLOC, C * 128], F32)
                nc.scalar.dma_start(u_nn[:], un_d.rearrange("n c f -> n (c f)"))
                mu_t = nsp.tile([NLOC, 128], F32)
                s2_t = nsp.tile([NLOC, 128], F32)
                tmp_t = nsp.tile([NLOC, 128], F32)
                un_v = u_nn[:].rearrange("n (c f) -> n f c", f=128)
                nc.vector.tensor_reduce(mu_t[:], un_v,
                                        axis=mybir.AxisListType.X, op=OP.add)
                zb_t = nsp.tile([NLOC, 1], F32)
                nc.vector.memset(zb_t[:], 0.0)
                sq_t = nsp.tile([NLOC, 16 * C], F32)
                sq_v = sq_t[:].rearrange("n (f c) -> n f c", f=16)
                for fc in range(8):
                    fsl = slice(fc * 16, (fc + 1) * 16)
                    nc.scalar.activation(sq_v, un_v[:, fsl, :], AF.Square,
                                         bias=zb_t[:, 0:1])
                    nc.vector.tensor_reduce(s2_t[:, fsl], sq_v,
                                            axis=mybir.AxisListType.X, op=OP.add)
                nc.vector.tensor_scalar_mul(mu_t[:], mu_t[:], 1.0 / C)
                nc.vector.tensor_scalar_mul(s2_t[:], s2_t[:], 1.0 / C)
                nc.vector.tensor_mul(tmp_t[:], mu_t[:], mu_t[:])
                nc.vector.tensor_sub(s2_t[:], s2_t[:], tmp_t[:])  # var
                eps_t = nsp.tile([NLOC, 1], F32)
                nc.vector.memset(eps_t[:], EPS)
                nc.scalar.activation(tmp_t[:], s2_t[:], AF.Sqrt,
                                     bias=eps_t[:, 0:1])
                nc.vector.reciprocal(A_t[:], tmp_t[:])            # rstd
                nc.vector.scalar_tensor_tensor(
                    B_t[:], mu_t[:], -1.0, A_t[:], op0=OP.mult, op1=OP.mult)

            # ================= norm apply (chunked) =================
            CH = 1024
            u_f = u_d.rearrange("c n f -> c (n f)")
            with (
                tc.tile_pool(name="nab", bufs=1) as nab,
                tc.tile_pool(name="nck", bufs=2) as nck,
                tc.tile_pool(name="npp", bufs=2, space="PSUM") as npp,
            ):
                for ch in range(NF // CH):
                    if ch % 4 == 0:
                        a1 = nab.tile([1, NF // 2], F32, tag="a1")
                        b1 = nab.tile([1, NF // 2], F32, tag="b1")
                        nr = slice((ch // 4) * 32, (ch // 4) * 32 + 32)
                        nc.sync.dma_start(a1[:], A_t[nr, :])
                        nc.sync.dma_start(b1[:], B_t[nr, :])
                    u_ck = nck.tile([C, CH], F32, tag="u")
                    nc.sync.dma_start(u_ck[:], u_f[:, ch * CH:(ch + 1) * CH])
                    ag = npp.tile([C, CH], F32, tag="ag")
                    bg = npp.tile([C, CH], F32, tag="bg")
                    for h2 in range(CH // 512):
                        lo = (ch % 4) * CH + h2 * 512
                        nc.tensor.matmul(ag[:, h2 * 512:(h2 + 1) * 512], gm1_t[:],
                                         a1[:, lo:lo + 512], start=True, stop=True)
                        nc.tensor.matmul(bg[:, h2 * 512:(h2 + 1) * 512], gm1_t[:],
                                         b1[:, lo:lo + 512], start=True, stop=True)
                    # xn chunk -> packed xn_t (rows 64:128 via ACT copy to
                    # keep all DVE ops at base partition 0)
                    csl = slice((ch % 4) * CH, (ch % 4) * CH + CH)
                    if ch < 4:
                        xv = xn_t[0:64, csl]
                        nc.vector.tensor_mul(xv, u_ck[:], ag[:])
                        nc.vector.scalar_tensor_tensor(
                            xv, xv, bt_t[:, 0:1], bg[:], op0=OP.add, op1=OP.add)
                    else:
                        xv = nck.tile([C, CH], F32, tag="xs")
                        nc.vector.tensor_mul(xv[:], u_ck[:], ag[:])
                        nc.vector.scalar_tensor_tensor(
                            xv[:], xv[:], bt_t[:, 0:1], bg[:],
                            op0=OP.add, op1=OP.add)
                        nc.scalar.activation(xn_t[64:128, csl], xv[:], AF.Copy)
                        xv = xv[:]
                    # fp8 copies into xn8 block A (s=0 rows 0:64, s=1 rows
                    # 64:128 shifted left by 1)
                    W0c = ch * CH
                    nc.scalar.activation(xn8_v[0:64, 0, W0c:W0c + CH], xv, AF.Copy)
                    if ch == 0:
                        nc.scalar.activation(xn8_v[64:128, 0, 0:CH - 1],
                                             xn_t[0:64, 1:CH], AF.Copy)
                    else:
                        nc.scalar.activation(xn8_v[64:128, 0, W0c - 1:W0c + CH - 1],
                                             xv, AF.Copy)
                    # block B = block A shifted left by 2 (DVE fp8 copy)
                    bdst = max(0, W0c - 2)
                    nc.vector.tensor_copy(xn8_v[:, 1, bdst:W0c + CH - 2],
                                          xn8_v[:, 0, bdst + 2:W0c + CH])

            # ================= SRU layers =================
            sig = AF.Sigmoid
            with (
                tc.tile_pool(name="gates", bufs=2) as gp,
                tc.tile_pool(name="gateb", bufs=1) as gb,
                tc.tile_pool(name="lps", bufs=2, space="PSUM") as pp,
            ):
                for li in range(4):
                    hout = [h_t[2 * (li % 2)], h_t[2 * (li % 2) + 1]]
                    hin = [h_t[2 * ((li - 1) % 2)], h_t[2 * ((li - 1) % 2) + 1]]
                    h8in = h8_v[(li - 1) % 2]
                    h8out = h8_v[li % 2]
                    ooff = 8 if li == 3 else 0
                    usc = 1.0 / WS
                    for half in range(2):
                        ubase = half * (NF // 2)
                        for d in range(2):
                            bcol = bfp_t[:, 2 * li + d:2 * li + d + 1]
                            rcol = brp_t[:, 2 * li + d:2 * li + d + 1]
                            f_t = gp.tile([128, NF // 2], BF16, tag="f")
                            r_t = gp.tile([128, NF // 2], BF16, tag="r")
                            w_t = gp.tile([128, NF // 2], BF16, tag="w")
                            b_t = gb.tile([128, NF // 2], BF16, tag="b")
                            f_v = f_t[:].rearrange("p (n l) -> p n l", l=128)
                            b_v = b_t[:].rearrange("p (n l) -> p n l", l=128)

                            # ---- phase A: f,r matmuls + sigmoid evac ----
                            for s in range(4):
                                fA = pp.tile([128, SPAN], F32, tag="g0")
                                rA = pp.tile([128, SPAN], F32, tag="g1")
                                for oi, ps in ((1, fA), (2, rA)):
                                    for b2 in range(2):
                                        osl = ps[:, b2 * 512:(b2 + 1) * 512]
                                        base = ubase + s * SPAN + b2 * 512
                                        if li == 0:
                                            for j in range(2):
                                                nc.tensor.matmul(
                                                    osl, w0_v[:, d, j, :, oi, :],
                                                    xn8_v[:, :, base + 4 * j:base + 4 * j + 512],
                                                    start=(j == 0), stop=(j == 1),
                                                    perf_mode=DR)
                                        else:
                                            nc.tensor.matmul(
                                                osl, wfr_v[:, li - 1, d, :, oi - 1, :],
                                                h8in[:, :, base:base + 512],
                                                start=True, stop=True,
                                                perf_mode=DR)
                                ssl = slice(s * SPAN, (s + 1) * SPAN)
                                if d == 0:
                                    fsrc, rsrc = fA[:], rA[:]
                                else:
                                    fsrc = fA[:].rearrange("p (n l) -> p n l", l=128)[:, :, ::-1]
                                    rsrc = rA[:].rearrange("p (n l) -> p n l", l=128)[:, :, ::-1]
                                nc.scalar.activation(f_t[:, ssl], fsrc, sig,
                                                     bias=bcol, scale=usc)
                                nc.scalar.activation(r_t[:, ssl], rsrc, sig,
                                                     bias=rcol, scale=usc)

                            # ---- phase B: z,hp matmuls + fused evac ----
                            zsc = -usc if li == 0 else -1.0
                            zbi = usc if li == 0 else 1.0
                            for s in range(4):
                                zB = pp.tile([128, SPAN], F32, tag="g0")
                                hB = pp.tile([128, SPAN], F32, tag="g1")
                                for oi, ps in ((0, zB), (1, hB)):
                                    for b2 in range(2):
                                        osl = ps[:, b2 * 512:(b2 + 1) * 512]
                                        base = ubase + s * SPAN + b2 * 512
                                        if li == 0:
                                            o4 = 0 if oi == 0 else 3
                                            for j in range(2):
                                                nc.tensor.matmul(
                                                    osl, w0_v[:, d, j, :, o4, :],
                                                    xn8_v[:, :, base + 4 * j:base + 4 * j + 512],
                                                    start=(j == 0), stop=(j == 1),
                                                    perf_mode=DR)
                                        else:
                                            for ct in range(2):
                                                nc.tensor.matmul(
                                                    osl, wzh_v[:, li - 1, d, ct, oi, :],
                                                    hin[ct][:, base:base + 512],
                                                    start=(ct == 0), stop=(ct == 1))
                                ssl = slice(s * SPAN, (s + 1) * SPAN)
                                if d == 0:
                                    zsrc, hsrc = zB[:], hB[:]
                                else:
                                    zsrc = zB[:].rearrange("p (n l) -> p n l", l=128)[:, :, ::-1]
                                    hsrc = hB[:].rearrange("p (n l) -> p n l", l=128)[:, :, ::-1]
                                # b = (1-f) * z  (+ 1/32 unscale for li=0)
                                nc.vector.affine_mul_reduce(
                                    b_t[:, ssl], acc_t[:], f_t[:, ssl], zsrc,
                                    scale=zsc, bias=zbi)
                                if li == 0:
                                    nc.scalar.activation(w_t[:, ssl], hsrc,
                                                         AF.Identity, bias=0.0,
                                                         scale=usc)
                                else:
                                    nc.scalar.activation(w_t[:, ssl], hsrc, AF.Copy)

                            # pads reset the scan carry between sequences
                            pads = slice(121, 128) if d == 0 else slice(0, 7)
                            nc.gpsimd.memset(f_v[:, :, pads], 0.0)
                            nc.gpsimd.memset(b_v[:, :, pads], 0.0)
                            # c = f*c + b
                            nc.vector.tensor_tensor_scan(
                                b_t[:], f_t[:], b_t[:], 0.0,
                                op0=OP.mult, op1=OP.add)
                            # highway: out = r*(cs-hp) + hp
                            nc.vector.tensor_sub(f_t[:], b_t[:], w_t[:])
                            nc.vector.tensor_mul(r_t[:], r_t[:], f_t[:])
                            hov = hout[d][:, ubase:ubase + NF // 2].rearrange(
                                "p (n l) -> p n l", l=128)
                            dst = hov if d == 0 else hov[:, :, ::-1]
                            r_v = r_t[:].rearrange("p (n l) -> p n l", l=128)
                            w_v = w_t[:].rearrange("p (n l) -> p n l", l=128)
                            nc.gpsimd.tensor_add(dst, r_v, w_v)
                            # fp8 copy for next layer / conv
                            nc.scalar.activation(
                                h8out[:, d, ooff + ubase:ooff + ubase + NF // 2],
                                hout[d][:, ubase:ubase + NF // 2], AF.Copy)

            # ================= transposed conv + residual =================
            h8c = h8_v[1]
            # zero the gaps: cols [0:8), per-block junk l in [121,128) (in
            # +8 frame: cols (n+1)*128 + [1,8)), and the tail
            for g in range(2):
                nc.gpsimd.memset(h8c[:, g, 0:8], 0.0)
                jv = h8c[:, g, 8:8 + NF].rearrange("p (n l) -> p n l", l=128)
                nc.gpsimd.memset(jv[:, :, 121:128], 0.0)
                nc.gpsimd.memset(h8c[:, g, 8 + NF:HC8], 0.0)
            with (
                tc.tile_pool(name="cvp", bufs=4, space="PSUM") as cvp,
                tc.tile_pool(name="osp", bufs=2) as osp,
            ):
                for chunk in range(NF // 512):
                    c_ps = cvp.tile([C, 512], F32, tag="c")
                    base = chunk * 512
                    for k in range(8):
                        nc.tensor.matmul(
                            c_ps[:], cw_v[:, k, :, :],
                            h8c[:, :, 8 - k + base:8 - k + base + 512],
                            start=(k == 0), stop=(k == 7), perf_mode=DR)
                    o_t = osp.tile([128, 512], F32, tag="o")
                    rsl = slice(0, 64) if chunk < 8 else slice(64, 128)
                    csl = slice((chunk % 8) * 512, (chunk % 8) * 512 + 512)
                    nc.vector.scalar_tensor_tensor(
                        o_t[rsl, :], xn_t[rsl, csl], cb_t[rsl, 0:1], c_ps[:],
                        op0=OP.add, op1=OP.add)
                    nc.sync.dma_start(out_d[:, base:base + 512], o_t[rsl, :])

    nc.compile()
    return nc


def _prep_weights(W0, Ws, convW):
    # L0: DR lhsT per (d, j): row p=c+64s, group g -> W0[c*K + s+2g+4j, d, :]
    w0r = W0.reshape(C, K, 2, 4 * H)
    w0p = np.zeros((2, 2, 128, 2, 4, 128), np.float32)
    for dd in range(2):
        for j in range(2):
            for s in range(2):
                for g in range(2):
                    k = s + 2 * g + 4 * j
                    w0p[dd, j, s * 64:(s + 1) * 64, g] = (
                        w0r[:, k, dd].reshape(C, 4, 128))
    w0p = (w0p * WS).reshape(2, 2, 128, 2 * 4 * 128)
    # L1-3 f,r: DR lhsT per (li, d): group g = direction block of h
    wfr = np.zeros((3, 2, 128, 2, 2, 128), np.float32)
    for i in range(3):
        for dd in range(2):
            for g in range(2):
                wfr[i, dd, :, g] = Ws[i][g * 128:(g + 1) * 128, dd,
                                         H:3 * H].reshape(128, 2, 128)
    wfr = (wfr * WS).reshape(3, 2, 128, 2 * 2 * 128)
    # L1-3 z,hp: bf16 per (li, d, ct)
    wzh = np.zeros((3, 2, 2, 128, 2, 128), np.float32)
    for i in range(3):
        for dd in range(2):
            for ct in range(2):
                wzh[i, dd, ct, :, 0] = Ws[i][ct * 128:(ct + 1) * 128, dd, 0:H]
                wzh[i, dd, ct, :, 1] = Ws[i][ct * 128:(ct + 1) * 128, dd, 3 * H:4 * H]
    wzh = wzh.reshape(3, 2, 2, 128, 2 * 128)
    # conv: DR lhsT per k (unscaled)
    cwp = np.zeros((8, 128, 2, C), np.float32)
    for k in range(8):
        for g in range(2):
            cwp[k, :, g] = convW[g * 128:(g + 1) * 128, :, k]
    cwp = cwp.reshape(8, 128, 2 * C)
    return (w0p.astype(E4), wfr.astype(E4), wzh.astype(BFD), cwp.astype(E4))


def kernel(**inputs):
    inputs = {k: np.asarray(v) for k, v in inputs.items()}
    x = inputs["x"].astype(np.float32)
    xs = np.ascontiguousarray(
        x.transpose(0, 2, 1, 3).reshape(B * T, C, F_))  # (512, C, F)

    w0p, wfr, wzh, cwp = _prep_weights(
        inputs["W0"].astype(np.float32),
        [inputs[f"W{i}"].astype(np.float32) for i in (1, 2, 3)],
        inputs["convW"].astype(np.float32))
    bfp = np.stack([inputs[f"bf{i}"] for i in range(4)]).astype(np.float32)
    brp = np.stack([inputs[f"br{i}"] for i in range(4)]).astype(np.float32)
    gm = inputs["gamma"].reshape(C).astype(np.float32)
    bt = inputs["beta"].reshape(C).astype(np.float32)
    cb = np.concatenate([inputs["convb"].reshape(C)] * 2).astype(np.float32)

    if "nc" not in _CACHE:
        _CACHE["nc"] = _build()
    nc = _CACHE["nc"]

    shared = {"w0p": w0p, "wfr": wfr, "wzh": wzh, "cwp": cwp, "bfp": bfp,
              "brp": brp, "gm": gm, "bt": bt, "cb": cb}
    in_maps = []
    for core in range(NCORES):
        sh = xs[core * NLOC:(core + 1) * NLOC]  # (NLOC, C, F)
        u = np.ascontiguousarray(sh.transpose(1, 0, 2))  # (C, NLOC, F)
        un = np.ascontiguousarray(sh)
        in_maps.append({"u": u, "un": un, **shared})

    trace = bool(os.environ.get("KBENCH_TRACE"))
    res = bass_utils.run_bass_kernel_spmd(
        nc, in_maps, list(range(NCORES)), trace=trace,
        tmpdir=os.environ.get("KBENCH_TMPDIR"))
    _CACHE["last_result"] = res

    full = np.concatenate(
        [res.results[i]["o"].reshape(C, NLOC, F_) for i in range(NCORES)],
        axis=1)  # (C, 512, F)
    out = full.transpose(1, 0, 2).reshape(B, T, C, F_).transpose(0, 2, 1, 3)
    return np.ascontiguousarray(out.astype(np.float32))
